# revision 24
# baseline (speedup 1.0000x reference)
"""Trainium2 Bass kernel for nn_AttentionDecoder (Bahdanau attention + GRU greedy decoder).

Sharding: pure data parallel, B=2048 split as 256 rows per core across 8 cores.

v4 design:
  - enc_bd / att / GRU+dec weights in fp8 E3M4 (weights pre-scaled x32 on host,
    un-scaled on PSUM evacuation): stationary LDWEIGHTS streams get fp8 FWL.
  - context mean-centered (attn - 1/32, meanT added on evacuation) so fp8 enc
    error applies only to the attention deviation.
  - the decode step is a 2-half software pipeline: attention, GRU gates,
    elementwise tail, fc2/argmax all emitted per 128-row half so each engine
    works one half while another engine works the other half.
  - PSUM banks host interleaved accumulation groups: only the first matmul of
    a bank per step uses start=True (clears the whole bank's has_written
    bits); later matmuls use start=False (first touch of a region overwrites,
    later touches accumulate).
  - biases ride ACT bias/scale operands (per-partition = per-feature);
    sigmoid via tanh(x/2); softmax without max-subtraction, Z via accum_out.
  - prologue: host supplies enc8 already in bd layout (one contiguous DMA);
    mean via a log2 tree of DVE adds; xbar transposes serial on the sync queue
    (concurrent transposes on both queues corrupt data).
"""

import os
import threading
import numpy as np
import ml_dtypes

N_CORES = 8
B, T, ENC = 2048, 32, 512
DEC, ATT, EMB, NCLS, L = 256, 256, 64, 37, 10
BL = B // N_CORES  # 256 per core
WS = 32.0  # fp8 weight pre-scale

_BF = ml_dtypes.bfloat16
_F8 = ml_dtypes.float8_e3m4

_lock = threading.Lock()
_cache = {}


def _build():
    import concourse.bass as bass
    import concourse.tile as tile
    from concourse import bacc, mybir

    bf = mybir.dt.bfloat16
    f8 = mybir.dt.float8e3
    f32 = mybir.dt.float32

    nc = bacc.Bacc("TRN2", target_bir_lowering=False, debug=False,
                   num_devices=N_CORES)

    # ---------------- DRAM inputs ----------------
    d_enc = nc.dram_tensor("enc", [2, T, 128, ENC], bf, kind="ExternalInput").ap()
    d_enc8 = nc.dram_tensor("enc8bd", [128, 64, ENC // 2], bf, kind="ExternalInput").ap()
    d_wdec = nc.dram_tensor("w_dec", [DEC, ATT], f8, kind="ExternalInput").ap()
    d_wenc = nc.dram_tensor("w_enc", [ENC, ATT], bf, kind="ExternalInput").ap()
    d_v = nc.dram_tensor("v", [ATT, 1], bf, kind="ExternalInput").ap()
    d_embW = nc.dram_tensor("embW", [NCLS, 3 * DEC], bf, kind="ExternalInput").ap()
    d_wihc = nc.dram_tensor("w_ih_c", [ENC, 3 * DEC], f8, kind="ExternalInput").ap()
    d_whhrz = nc.dram_tensor("w_hh_rz", [DEC, 2 * DEC], f8, kind="ExternalInput").ap()
    d_whhn2 = nc.dram_tensor("w_hh_n2", [DEC, DEC], f8, kind="ExternalInput").ap()
    d_bias_rz2 = nc.dram_tensor("bias_rz2", [128, 4], f32, kind="ExternalInput").ap()
    d_bias_n = nc.dram_tensor("bias_n", [128, 2], f32, kind="ExternalInput").ap()
    d_bhhn2 = nc.dram_tensor("bhhn2", [1, 2, 128], bf, kind="ExternalInput").ap()
    d_fc1h = nc.dram_tensor("fc1_w_h", [DEC, DEC], bf, kind="ExternalInput").ap()
    d_fc1c = nc.dram_tensor("fc1_w_c", [ENC, DEC], bf, kind="ExternalInput").ap()
    d_fc1b = nc.dram_tensor("fc1_b", [128, 2], f32, kind="ExternalInput").ap()
    d_fc2w = nc.dram_tensor("fc2_w", [DEC, NCLS], bf, kind="ExternalInput").ap()
    d_fc2b = nc.dram_tensor("fc2_b", [1, NCLS], bf, kind="ExternalInput").ap()
    d_ihw = nc.dram_tensor("init_h_w", [ENC, DEC], bf, kind="ExternalInput").ap()
    d_ihb = nc.dram_tensor("init_h_b", [128, 2], f32, kind="ExternalInput").ap()
    d_out = nc.dram_tensor("out", [BL, L, NCLS], f32, kind="ExternalOutput").ap()

    ident_np = np.eye(128, dtype=_BF)
    d_ident = nc.inline_tensor(ident_np, name="ident").ap()
    d_onesrow = nc.inline_tensor(np.ones((1, 256), dtype=_BF), name="onesrow").ap()

    AluOp = mybir.AluOpType
    ActF = mybir.ActivationFunctionType
    AxX = mybir.AxisListType.X
    INV = 1.0 / WS

    with tile.TileContext(nc) as tc:
        with (
            tc.tile_pool(name="persist", bufs=1) as P,
            tc.tile_pool(name="wpool", bufs=1) as W,
            tc.tile_pool(name="trans", bufs=1) as TR,
            tc.tile_pool(name="small", bufs=1) as SM,
            tc.tile_pool(name="psG", bufs=2, space="PSUM") as PG,
            tc.tile_pool(name="psB", bufs=2, space="PSUM") as PB,
        ):
            # ---------------- persistent SBUF ----------------
            enc_bd_raw = P.tile([128, 64, ENC // 2], bf, tag="enc_bd")  # fp8 bytes, 32KB/part
            ep = P.tile([128, 2, T, 256], bf, tag="ep")            # 32KB
            att = P.tile([128, 2, T, 256], f8, tag="att")          # 16KB
            hT = P.tile([128, 2, BL], bf, tag="hT")
            ctxT = P.tile([128, 4, BL], bf, tag="ctxT")
            meanT = P.tile([128, 4, BL], bf, tag="meanT")
            onehotT = P.tile([NCLS, BL], bf, tag="onehotT")
            attn_bd = P.tile([128, 64, 4], bf, tag="attn_bd")
            out_sb = P.tile([128, 2, L, NCLS], f32, tag="out_sb")

            # ---------------- weights ----------------
            def wload(tag, shape, src, rearr=None, q=nc.scalar, dt=bf):
                t = W.tile(shape, dt, tag=tag)
                q.dma_start(t[:], src if rearr is None else src.rearrange(rearr, p=128))
                return t

            w_dec = wload("w_dec", [128, 2, ATT], d_wdec, "(k p) n -> p k n", dt=f8)
            w_enc = wload("w_enc", [128, 4, ATT], d_wenc, "(k p) n -> p k n")
            v_sb = wload("v_sb", [128, 2, 1], d_v, "(k p) n -> p k n")
            embW = wload("embW", [NCLS, 3 * DEC], d_embW)
            w_ihc = wload("w_ihc", [128, 4, 3 * DEC], d_wihc, "(k p) n -> p k n",
                          dt=f8)
            w_hhrz = wload("w_hhrz", [128, 2, 2 * DEC], d_whhrz, "(k p) n -> p k n",
                           dt=f8)
            w_hhn2 = wload("w_hhn2", [128, 2, DEC], d_whhn2, "(k p) n -> p k n",
                           dt=f8)
            bhhn2 = wload("bhhn2", [1, 2, 128], d_bhhn2)
            fc1h = wload("fc1h", [128, 2, DEC], d_fc1h, "(k p) n -> p k n")
            fc1c = wload("fc1c", [128, 4, DEC], d_fc1c, "(k p) n -> p k n")
            fc2w = wload("fc2w", [128, 2, NCLS], d_fc2w, "(k p) n -> p k n")
            fc2b = wload("fc2b", [1, NCLS], d_fc2b)
            ihw = wload("ihw", [128, 4, DEC], d_ihw, "(k p) n -> p k n")
            ident = wload("ident", [128, 128], d_ident)
            ones1 = W.tile([1, 128], bf)
            nc.scalar.dma_start(ones1[:], d_onesrow[:, 0:128])
            onesN = W.tile([1, BL], bf)
            nc.scalar.dma_start(onesN[:], d_onesrow[:])

            def f32load(tag, shape, src, q=nc.scalar):
                t = W.tile(shape, f32, tag=tag)
                q.dma_start(t[:], src)
                return t

            bias_rz2 = f32load("bias_rz2", [128, 4], d_bias_rz2)
            bias_n = f32load("bias_n", [128, 2], d_bias_n)
            fc1b = f32load("fc1b", [128, 2], d_fc1b)
            ihb = f32load("ihb", [128, 2], d_ihb)

            # ---------------- prologue ----------------
            nc.vector.memset(onehotT[:], 0)
            nc.vector.memset(onehotT[0:1, :], 1.0)
            nc.vector.memset(attn_bd[:], 0)

            # enc_bd: host pre-laid-out, one contiguous DMA on the scalar queue
            for i in range(4):
                nc.scalar.dma_start(enc_bd_raw[:, 16 * i:16 * i + 16, :],
                                    d_enc8[:, 16 * i:16 * i + 16, :])
            enc_bd = enc_bd_raw[:].bitcast(f8)

            encTh = TR.tile([128, 4, 32 * 128], bf, tag="encTh", bufs=1)
            for bth in range(2):
                d_encb = d_enc[bth].rearrange("t b e -> (t b) e")
                for eb in range(4):
                    nc.sync.dma_start_transpose(encTh[:, eb, :],
                                                d_encb[:, 128 * eb:128 * eb + 128])
                # mean over t via log2 tree of DVE adds (cols are t-major)
                for eb in range(4):
                    ms = SM.tile([128, 2048], bf, tag="ms")
                    nc.vector.tensor_tensor(ms[:], encTh[:, eb, 0:2048],
                                            encTh[:, eb, 2048:4096], op=AluOp.add)
                    for w in (1024, 512, 256, 128):
                        nc.vector.tensor_tensor(ms[:, 0:w], ms[:, 0:w],
                                                ms[:, w:2 * w], op=AluOp.add)
                    nc.vector.tensor_scalar(
                        meanT[:, eb, 128 * bth:128 * bth + 128], ms[:, 0:128],
                        1.0 / 32.0, None, op0=AluOp.mult)
                # enc_proj: ep[a, t, b] = W_enc^T @ encT
                for c in range(8):
                    for ab in range(2):
                        pp = PG.tile([128, 512], f32, tag="rz")
                        for eb in range(4):
                            nc.tensor.matmul(
                                pp[:],
                                w_enc[:, eb, 128 * ab:128 * ab + 128],
                                encTh[:, eb, 512 * c:512 * c + 512],
                                start=(eb == 0), stop=(eb == 3),
                            )
                        dst = ep[:, ab, 4 * c:4 * c + 4, 128 * bth:128 * bth + 128]
                        src2 = pp[:].rearrange("p (t b) -> p t b", b=128)
                        if ab == 0:
                            nc.vector.tensor_copy(dst, src2)
                        else:
                            nc.scalar.copy(dst, src2)

            # h0 = tanh(meanT @ ihw + ihb), feature-major
            for db in range(2):
                hp = PG.tile([128, BL], f32, tag="ng")
                for eb in range(4):
                    nc.tensor.matmul(hp[:], ihw[:, eb, 128 * db:128 * db + 128],
                                     meanT[:, eb, :], start=(eb == 0), stop=(eb == 3))
                nc.scalar.activation(hT[:, db, :], hp[:], ActF.Tanh,
                                     bias=ihb[:, db:db + 1])

            # ---------------- decode loop (2-half software pipeline) ----------------
            pending_oh = [None, None]  # ohB tiles awaiting transpose into onehotT
            for step in range(L):
                # dec_proj per half (fp8 w_dec x32)
                dps = []
                for half in range(2):
                    hs = 128 * half
                    dp = PG.tile([128, 2, 128], f32, tag="cp")
                    first = True
                    for db in range(2):
                        for ab in range(2):
                            nc.tensor.matmul(dp[:, ab, :],
                                             w_dec[:, db, 128 * ab:128 * ab + 128],
                                             hT[:, db, hs:hs + 128],
                                             start=first, stop=(db == 1 and ab == 1),
                                             skip_group_check=True)
                            first = False
                    dps.append(dp)

                # pending argmax transposes from previous step
                for half in range(2):
                    if pending_oh[half] is not None:
                        hs = 128 * half
                        tp2 = PB.tile([NCLS, 128], bf, tag="b")
                        nc.tensor.transpose(tp2[:], pending_oh[half][:], ident[:])
                        nc.vector.tensor_copy(onehotT[:, hs:hs + 128], tp2[:])
                        pending_oh[half] = None
                decTs = []
                for half in range(2):
                    decT = SM.tile([128, 2, 128], bf, tag=f"decT{half}")
                    nc.vector.tensor_scalar(decT[:], dps[half][:], INV, None,
                                            op0=AluOp.mult)
                    decTs.append(decT)

                # attention adds (DVE)
                stmps = []
                for half in range(2):
                    hs = 128 * half
                    stmp = SM.tile([128, 2, T, 128], bf, tag=f"stmp{half}")
                    for ab in range(2):
                        bcast = decTs[half][:, ab, :].rearrange(
                            "p (o b) -> p o b", o=1).broadcast_to([128, T, 128])
                        nc.vector.tensor_tensor(stmp[:, ab], ep[:, ab, :, hs:hs + 128],
                                                bcast, op=AluOp.add)
                    stmps.append(stmp)

                # GRU gate banks; early (emb + h) parts
                # rz bank cols: [r0, r1, z0, z1]; ng bank cols: [n0, n1, g0, g1]
                rzs, ngs = [None, None], [None, None]

                def gates_early(half):
                    hs = 128 * half
                    rz = PG.tile([128, 4, 128], f32, tag="rz")
                    ng = PG.tile([128, 4, 128], f32, tag="ng")
                    first = True
                    for ci in range(4):
                        fs = 128 * ci
                        nc.tensor.matmul(rz[:, ci, :], embW[:, fs:fs + 128],
                                         onehotT[:, hs:hs + 128],
                                         start=first, stop=False,
                                         skip_group_check=True)
                        first = False
                        for db in range(2):
                            nc.tensor.matmul(rz[:, ci, :], w_hhrz[:, db, fs:fs + 128],
                                             hT[:, db, hs:hs + 128],
                                             start=False, stop=False,
                                             skip_group_check=True)
                    first = True
                    for ci in range(2):
                        fs = 512 + 128 * ci
                        nc.tensor.matmul(ng[:, ci, :], embW[:, fs:fs + 128],
                                         onehotT[:, hs:hs + 128],
                                         start=first, stop=False,
                                         skip_group_check=True)
                        first = False
                    for ci in range(2):
                        nc.tensor.matmul(ng[:, 2 + ci, :], bhhn2[:, ci, :],
                                         onesN[:, 0:128], start=False, stop=False,
                                         skip_group_check=True)
                        for db in range(2):
                            nc.tensor.matmul(ng[:, 2 + ci, :],
                                             w_hhn2[:, db, 128 * ci:128 * ci + 128],
                                             hT[:, db, hs:hs + 128],
                                             start=False,
                                             stop=(ci == 1 and db == 1),
                                             skip_group_check=True)
                    rzs[half] = rz
                    ngs[half] = ng

                # attention phases, split for explicit ACT/PE queue ordering
                ebps, attnBs = {}, {}

                def a_tanh(half, ab):
                    hs = 128 * half
                    nc.scalar.activation(att[:, ab, :, hs:hs + 128],
                                         stmps[half][:, ab], ActF.Tanh)

                def a_energy(half):
                    hs = 128 * half
                    ebp = PB.tile([128, T], f32, tag="b")
                    for t in range(T):
                        for ab in range(2):
                            nc.tensor.matmul(ebp[:, t:t + 1],
                                             att[:, ab, t, hs:hs + 128],
                                             v_sb[:, ab, :],
                                             start=(t == 0 and ab == 0),
                                             stop=(t == T - 1 and ab == 1),
                                             skip_group_check=True)
                    ebps[half] = ebp

                def a_soft(half):
                    expB = SM.tile([128, T], bf, tag=f"expB{half}")
                    zc = SM.tile([128, 1], f32, tag=f"zc{half}")
                    nc.scalar.activation(expB[:], ebps[half][:], ActF.Exp,
                                         accum_out=zc[:])
                    rcp = SM.tile([128, 1], f32, tag=f"rcp{half}")
                    nc.vector.reciprocal(rcp[:], zc[:])
                    attnB = SM.tile([128, T], bf, tag=f"attnB{half}")
                    nc.vector.tensor_scalar(attnB[:], expB[:], rcp[:], -1.0 / T,
                                            op0=AluOp.mult, op1=AluOp.add)
                    tp = PB.tile([T, 128], bf, tag="b")
                    nc.tensor.transpose(tp[:], attnB[:], ident[:])
                    for bs in range(4):
                        nc.vector.tensor_copy(
                            attn_bd[32 * bs:32 * bs + 32,
                                    32 * half:32 * half + 32, bs],
                            tp[:, bs::4])

                def a_ctx(half):
                    hs = 128 * half
                    cp = PG.tile([128, 4, 128], f32, tag="cp")
                    first = True
                    for eb in range(4):
                        for gr in range(32):
                            g = 32 * half + gr
                            nc.tensor.matmul(cp[:, eb, 4 * gr:4 * gr + 4],
                                             enc_bd[:, g, 128 * eb:128 * eb + 128],
                                             attn_bd[:, g, :], start=first,
                                             stop=(eb == 3 and gr == 31),
                                             skip_group_check=True)
                            first = False
                        nc.vector.tensor_tensor(ctxT[:, eb, hs:hs + 128],
                                                cp[:, eb, :],
                                                meanT[:, eb, hs:hs + 128],
                                                op=AluOp.add)
                    for ci in range(4):
                        fs = 128 * ci
                        for eb in range(4):
                            nc.tensor.matmul(rzs[half][:, ci, :],
                                             w_ihc[:, eb, fs:fs + 128],
                                             ctxT[:, eb, hs:hs + 128],
                                             start=False,
                                             stop=(ci == 3 and eb == 3),
                                             skip_group_check=True)
                    for ci in range(2):
                        fs = 512 + 128 * ci
                        for eb in range(4):
                            nc.tensor.matmul(ngs[half][:, ci, :],
                                             w_ihc[:, eb, fs:fs + 128],
                                             ctxT[:, eb, hs:hs + 128],
                                             start=False,
                                             stop=(ci == 1 and eb == 3),
                                             skip_group_check=True)

                gates_early(0)
                a_tanh(0, 0)
                a_tanh(0, 1)
                a_energy(0)
                gates_early(1)
                a_tanh(1, 0)
                a_soft(0)
                a_tanh(1, 1)
                a_ctx(0)
                a_energy(1)
                a_soft(1)
                a_ctx(1)

                # fc1 ctx-part early: fills the GRU-tail PE window
                bankF = PG.tile([128, 2, BL], f32, tag="rz")
                first = True
                for dc in range(2):
                    ds = 128 * dc
                    for eb in range(4):
                        nc.tensor.matmul(bankF[:, dc, :], fc1c[:, eb, ds:ds + 128],
                                         ctxT[:, eb, :], start=first, stop=False,
                                         skip_group_check=True)
                        first = False

                # GRU elementwise tail: pass 1 (tr/tz + n-preact) both halves
                npres, tzs = [], []
                for half in range(2):
                    rz, ng = rzs[half], ngs[half]
                    tr_sb = SM.tile([128, 2, 128], bf, tag=f"tr{half}")
                    tz_sb = SM.tile([128, 2, 128], bf, tag=f"tz{half}")
                    for ci in range(2):
                        nc.scalar.activation(tr_sb[:, ci, :], rz[:, ci, :], ActF.Tanh,
                                             bias=bias_rz2[:, ci:ci + 1],
                                             scale=0.5 * INV)
                    for ci in range(2):
                        nc.scalar.activation(tz_sb[:, ci, :], rz[:, 2 + ci, :],
                                             ActF.Tanh,
                                             bias=bias_rz2[:, 2 + ci:3 + ci],
                                             scale=0.5 * INV)
                    u_sb = SM.tile([128, 2, 128], bf, tag=f"u{half}")
                    nc.vector.tensor_scalar(u_sb[:], ng[:, 2:4, :], INV, None,
                                            op0=AluOp.mult)
                    w1 = SM.tile([128, 2, 128], bf, tag=f"w1{half}")
                    nc.vector.scalar_tensor_tensor(w1[:], tr_sb[:], 1.0, u_sb[:],
                                                   op0=AluOp.add, op1=AluOp.mult)
                    npre = SM.tile([128, 2, 128], bf, tag=f"np{half}")
                    nc.vector.scalar_tensor_tensor(npre[:], ng[:, 0:2, :], INV, w1[:],
                                                   op0=AluOp.mult, op1=AluOp.add)
                    npres.append(npre)
                    tzs.append(tz_sb)
                # pass 2: n + h update both halves
                for half in range(2):
                    hs = 128 * half
                    n_sb = SM.tile([128, 2, 128], bf, tag=f"n{half}")
                    for ci in range(2):
                        nc.scalar.activation(n_sb[:, ci, :], npres[half][:, ci, :],
                                             ActF.Tanh, bias=bias_n[:, ci:ci + 1])
                    t1 = SM.tile([128, 2, 128], bf, tag=f"t1{half}")
                    nc.vector.tensor_tensor(t1[:], hT[:, :, hs:hs + 128], n_sb[:],
                                            op=AluOp.subtract)
                    t3 = SM.tile([128, 2, 128], bf, tag=f"t3{half}")
                    nc.vector.scalar_tensor_tensor(t3[:], tzs[half][:], 1.0, t1[:],
                                                   op0=AluOp.add, op1=AluOp.mult)
                    nc.vector.scalar_tensor_tensor(hT[:, :, hs:hs + 128], t3[:], 0.5,
                                                   n_sb[:],
                                                   op0=AluOp.mult, op1=AluOp.add)

                # fc1 h-part (closes the accumulation)
                for dc in range(2):
                    ds = 128 * dc
                    for db in range(2):
                        nc.tensor.matmul(bankF[:, dc, :], fc1h[:, db, ds:ds + 128],
                                         hT[:, db, :], start=False,
                                         stop=(dc == 1 and db == 1),
                                         skip_group_check=True)
                hidT = SM.tile([128, 2, BL], bf, tag="hidT")
                for dc in range(2):
                    nc.vector.tensor_scalar(hidT[:, dc, :], bankF[:, dc, :],
                                            fc1b[:, dc:dc + 1], 0.0,
                                            op0=AluOp.add, op1=AluOp.max)

                # fc2 + greedy argmax per half
                for half in range(2):
                    hs = 128 * half
                    lp = PB.tile([128, NCLS], f32, tag="b")
                    nc.tensor.matmul(lp[:], ones1[:], fc2b[:], start=True, stop=False,
                                     skip_group_check=True)
                    for db in range(2):
                        nc.tensor.matmul(lp[:], hidT[:, db, hs:hs + 128],
                                         fc2w[:, db, :],
                                         start=False, stop=(db == 1),
                                         skip_group_check=True)
                    nc.scalar.copy(out_sb[:, half, step, :], lp[:])
                    if step < L - 1:
                        sl = out_sb[:, half, step, :]
                        mx = SM.tile([128, 1], f32, tag=f"mx{half}")
                        nc.vector.tensor_reduce(mx[:], sl, axis=AxX, op=AluOp.max)
                        ohB = SM.tile([128, NCLS], bf, tag=f"oh{half}")
                        nc.vector.tensor_tensor(
                            ohB[:], sl, mx[:].broadcast_to([128, NCLS]),
                            op=AluOp.is_equal)
                        pending_oh[half] = ohB

            # ---------------- output DMA ----------------
            for half in range(2):
                nc.sync.dma_start(
                    d_out[128 * half:128 * half + 128],
                    out_sb[:, half, :, :],
                )

    nc.compile()
    return nc


def _get_nc():
    with _lock:
        if "nc" not in _cache:
            _cache["nc"] = _build()
        return _cache["nc"]


def kernel(**inputs):
    nc = _get_nc()
    from concourse.bass_utils import run_bass_kernel_spmd

    enc = np.ascontiguousarray(inputs["encoder_outputs"], dtype=np.float32)
    emb = inputs["emb"].astype(np.float32)
    W_enc = inputs["W_enc"].astype(np.float32)
    W_dec = inputs["W_dec"].astype(np.float32)
    v = inputs["v"].astype(np.float32)
    init_h_W = inputs["init_h_W"].astype(np.float32)
    init_h_b = inputs["init_h_b"].astype(np.float32)
    W_ih = inputs["W_ih"].astype(np.float32)
    b_ih = inputs["b_ih"].astype(np.float32)
    W_hh = inputs["W_hh"].astype(np.float32)
    b_hh = inputs["b_hh"].astype(np.float32)
    fc1_W = inputs["fc1_W"].astype(np.float32)
    fc1_b = inputs["fc1_b"].astype(np.float32)
    fc2_W = inputs["fc2_W"].astype(np.float32)
    fc2_b = inputs["fc2_b"].astype(np.float32)

    embW = emb @ W_ih[:EMB]  # [NCLS, 768]
    bias_rz = b_ih[:2 * DEC] + b_hh[:2 * DEC]

    bfc = lambda a: np.ascontiguousarray(a, dtype=_BF)
    f8c = lambda a: np.ascontiguousarray(np.asarray(a) * WS, dtype=_F8)
    f32c = lambda a: np.ascontiguousarray(a, dtype=np.float32)
    colview = lambda a, k: f32c(a.reshape(k, 128).T)
    shared = {
        "w_dec": f8c(W_dec),
        "w_enc": bfc(W_enc),
        "v": bfc(v.reshape(ATT, 1)),
        "embW": bfc(WS * embW),
        "w_ih_c": f8c(W_ih[EMB:]),
        "w_hh_rz": f8c(W_hh[:, :2 * DEC]),
        "w_hh_n2": f8c(0.5 * W_hh[:, 2 * DEC:]),
        "bias_rz2": colview(0.5 * bias_rz, 4),
        "bias_n": colview(b_ih[2 * DEC:], 2),
        "bhhn2": bfc((WS * 0.5 * b_hh[2 * DEC:]).reshape(1, 2, 128)),
        "fc1_w_h": bfc(fc1_W[:DEC]),
        "fc1_w_c": bfc(fc1_W[DEC:]),
        "fc1_b": colview(fc1_b, 2),
        "fc2_w": bfc(fc2_W),
        "fc2_b": bfc(fc2_b.reshape(1, NCLS)),
        "init_h_w": bfc(init_h_W),
        "init_h_b": colview(init_h_b, 2),
    }
    enc_bf = enc.astype(_BF)
    enc_f8 = enc.astype(_F8)
    in_maps = []
    for i in range(N_CORES):
        m = dict(shared)
        sh = enc_bf[i * BL:(i + 1) * BL]
        m["enc"] = np.ascontiguousarray(
            sh.reshape(2, 128, T, ENC).transpose(0, 2, 1, 3))
        # enc8 in bd layout: partition p = 32*(b%4) + t, free g = (bth, (b%128)//4)
        sh8 = enc_f8[i * BL:(i + 1) * BL]  # [256, 32, 512]
        m["enc8bd"] = np.ascontiguousarray(
            sh8.reshape(2, 32, 4, T, ENC).transpose(2, 3, 0, 1, 4).reshape(
                128, 64, ENC)).view(_BF)
        in_maps.append(m)

    res = run_bass_kernel_spmd(nc, in_maps, core_ids=list(range(N_CORES)),
                               trace=bool(int(os.environ.get("KTRACE", "0"))))
    out = np.concatenate([res.results[i]["out"] for i in range(N_CORES)], axis=0)
    kernel.last_results = res.results
    if bool(int(os.environ.get("KTRACE", "0"))):
        kernel.last_exec_time_ns = res.exec_time_ns
        kernel.last_profile = res.profile_json
    return out.astype(np.float32)


# revision 26
# speedup vs baseline: 1.0015x; 1.0015x over previous
"""Trainium2 Bass kernel for nn_AttentionDecoder (Bahdanau attention + GRU greedy decoder).

Sharding: pure data parallel, B=2048 split as 256 rows per core across 8 cores.

v4 design:
  - enc_bd / att / GRU+dec weights in fp8 E3M4 (weights pre-scaled x32 on host,
    un-scaled on PSUM evacuation): stationary LDWEIGHTS streams get fp8 FWL.
  - context mean-centered (attn - 1/32, meanT added on evacuation) so fp8 enc
    error applies only to the attention deviation.
  - the decode step is a 2-half software pipeline: attention, GRU gates,
    elementwise tail, fc2/argmax all emitted per 128-row half so each engine
    works one half while another engine works the other half.
  - PSUM banks host interleaved accumulation groups: only the first matmul of
    a bank per step uses start=True (clears the whole bank's has_written
    bits); later matmuls use start=False (first touch of a region overwrites,
    later touches accumulate).
  - biases ride ACT bias/scale operands (per-partition = per-feature);
    sigmoid via tanh(x/2); softmax without max-subtraction, Z via accum_out.
  - prologue: host supplies enc8 already in bd layout (one contiguous DMA);
    mean via a log2 tree of DVE adds; xbar transposes serial on the sync queue
    (concurrent transposes on both queues corrupt data).
"""

import os
import threading
import numpy as np
import ml_dtypes

N_CORES = 8
B, T, ENC = 2048, 32, 512
DEC, ATT, EMB, NCLS, L = 256, 256, 64, 37, 10
BL = B // N_CORES  # 256 per core
WS = 32.0  # fp8 weight pre-scale

_BF = ml_dtypes.bfloat16
_F8 = ml_dtypes.float8_e3m4

_lock = threading.Lock()
_cache = {}


def _build():
    import concourse.bass as bass
    import concourse.tile as tile
    from concourse import bacc, mybir

    bf = mybir.dt.bfloat16
    f8 = mybir.dt.float8e3
    f32 = mybir.dt.float32

    nc = bacc.Bacc("TRN2", target_bir_lowering=False, debug=False,
                   num_devices=N_CORES)

    # ---------------- DRAM inputs ----------------
    d_enc = nc.dram_tensor("enc", [2, T, 128, ENC], bf, kind="ExternalInput").ap()
    d_enc8 = nc.dram_tensor("enc8bd", [128, 64, ENC // 2], bf, kind="ExternalInput").ap()
    d_wdec = nc.dram_tensor("w_dec", [DEC, ATT], f8, kind="ExternalInput").ap()
    d_wenc = nc.dram_tensor("w_enc", [ENC, ATT], bf, kind="ExternalInput").ap()
    d_v = nc.dram_tensor("v", [ATT, 1], bf, kind="ExternalInput").ap()
    d_embW = nc.dram_tensor("embW", [NCLS, 3 * DEC], bf, kind="ExternalInput").ap()
    d_wihc = nc.dram_tensor("w_ih_c", [ENC, 3 * DEC], f8, kind="ExternalInput").ap()
    d_whhrz = nc.dram_tensor("w_hh_rz", [DEC, 2 * DEC], f8, kind="ExternalInput").ap()
    d_whhn2 = nc.dram_tensor("w_hh_n2", [DEC, DEC], f8, kind="ExternalInput").ap()
    d_bias_rz2 = nc.dram_tensor("bias_rz2", [128, 4], f32, kind="ExternalInput").ap()
    d_bias_n = nc.dram_tensor("bias_n", [128, 2], f32, kind="ExternalInput").ap()
    d_bhhn2 = nc.dram_tensor("bhhn2", [1, 2, 128], bf, kind="ExternalInput").ap()
    d_fc1h = nc.dram_tensor("fc1_w_h", [DEC, DEC], bf, kind="ExternalInput").ap()
    d_fc1c = nc.dram_tensor("fc1_w_c", [ENC, DEC], bf, kind="ExternalInput").ap()
    d_fc1b = nc.dram_tensor("fc1_b", [128, 2], f32, kind="ExternalInput").ap()
    d_fc2w = nc.dram_tensor("fc2_w", [DEC, NCLS], bf, kind="ExternalInput").ap()
    d_fc2b = nc.dram_tensor("fc2_b", [1, NCLS], bf, kind="ExternalInput").ap()
    d_ihw = nc.dram_tensor("init_h_w", [ENC, DEC], bf, kind="ExternalInput").ap()
    d_ihb = nc.dram_tensor("init_h_b", [128, 2], f32, kind="ExternalInput").ap()
    d_out = nc.dram_tensor("out", [BL, L, NCLS], f32, kind="ExternalOutput").ap()

    ident_np = np.eye(128, dtype=_BF)
    d_ident = nc.inline_tensor(ident_np, name="ident").ap()
    d_onesrow = nc.inline_tensor(np.ones((1, 256), dtype=_BF), name="onesrow").ap()

    AluOp = mybir.AluOpType
    ActF = mybir.ActivationFunctionType
    AxX = mybir.AxisListType.X
    INV = 1.0 / WS

    with tile.TileContext(nc) as tc:
        with (
            tc.tile_pool(name="persist", bufs=1) as P,
            tc.tile_pool(name="wpool", bufs=1) as W,
            tc.tile_pool(name="trans", bufs=1) as TR,
            tc.tile_pool(name="small", bufs=1) as SM,
            tc.tile_pool(name="psG", bufs=2, space="PSUM") as PG,
            tc.tile_pool(name="psB", bufs=2, space="PSUM") as PB,
        ):
            # ---------------- persistent SBUF ----------------
            enc_bd_raw = P.tile([128, 64, ENC // 2], bf, tag="enc_bd")  # fp8 bytes, 32KB/part
            ep = P.tile([128, 2, T, 256], bf, tag="ep")            # 32KB
            att = P.tile([128, 2, T, 256], f8, tag="att")          # 16KB
            hT = P.tile([128, 2, BL], bf, tag="hT")
            ctxT = P.tile([128, 4, BL], bf, tag="ctxT")
            meanT = P.tile([128, 4, BL], bf, tag="meanT")
            onehotT = P.tile([NCLS, BL], bf, tag="onehotT")
            attn_bd = P.tile([128, 64, 4], bf, tag="attn_bd")
            out_sb = P.tile([128, 2, L, NCLS], f32, tag="out_sb")

            # ---------------- weights ----------------
            def wload(tag, shape, src, rearr=None, q=nc.scalar, dt=bf):
                t = W.tile(shape, dt, tag=tag)
                q.dma_start(t[:], src if rearr is None else src.rearrange(rearr, p=128))
                return t

            w_dec = wload("w_dec", [128, 2, ATT], d_wdec, "(k p) n -> p k n", dt=f8)
            w_enc = wload("w_enc", [128, 4, ATT], d_wenc, "(k p) n -> p k n")
            v_sb = wload("v_sb", [128, 2, 1], d_v, "(k p) n -> p k n")
            embW = wload("embW", [NCLS, 3 * DEC], d_embW)
            w_ihc = wload("w_ihc", [128, 4, 3 * DEC], d_wihc, "(k p) n -> p k n",
                          dt=f8)
            w_hhrz = wload("w_hhrz", [128, 2, 2 * DEC], d_whhrz, "(k p) n -> p k n",
                           dt=f8)
            w_hhn2 = wload("w_hhn2", [128, 2, DEC], d_whhn2, "(k p) n -> p k n",
                           dt=f8)
            bhhn2 = wload("bhhn2", [1, 2, 128], d_bhhn2)
            fc1h = wload("fc1h", [128, 2, DEC], d_fc1h, "(k p) n -> p k n")
            fc1c = wload("fc1c", [128, 4, DEC], d_fc1c, "(k p) n -> p k n")
            fc2w = wload("fc2w", [128, 2, NCLS], d_fc2w, "(k p) n -> p k n")
            fc2b = wload("fc2b", [1, NCLS], d_fc2b)
            ihw = wload("ihw", [128, 4, DEC], d_ihw, "(k p) n -> p k n")
            ident = wload("ident", [128, 128], d_ident)
            ones1 = W.tile([1, 128], bf)
            nc.scalar.dma_start(ones1[:], d_onesrow[:, 0:128])
            onesN = W.tile([1, BL], bf)
            nc.scalar.dma_start(onesN[:], d_onesrow[:])

            def f32load(tag, shape, src, q=nc.scalar):
                t = W.tile(shape, f32, tag=tag)
                q.dma_start(t[:], src)
                return t

            bias_rz2 = f32load("bias_rz2", [128, 4], d_bias_rz2)
            bias_n = f32load("bias_n", [128, 2], d_bias_n)
            fc1b = f32load("fc1b", [128, 2], d_fc1b)
            ihb = f32load("ihb", [128, 2], d_ihb)

            # ---------------- prologue ----------------
            nc.vector.memset(onehotT[:], 0)
            nc.vector.memset(onehotT[0:1, :], 1.0)
            nc.vector.memset(attn_bd[:], 0)

            # enc_bd: host pre-laid-out, one contiguous DMA on the scalar queue
            for i in range(4):
                nc.scalar.dma_start(enc_bd_raw[:, 16 * i:16 * i + 16, :],
                                    d_enc8[:, 16 * i:16 * i + 16, :])
            enc_bd = enc_bd_raw[:].bitcast(f8)

            encTh = TR.tile([128, 4, 32 * 128], bf, tag="encTh", bufs=1)
            for bth in range(2):
                d_encb = d_enc[bth].rearrange("t b e -> (t b) e")
                for eb in range(4):
                    nc.sync.dma_start_transpose(encTh[:, eb, :],
                                                d_encb[:, 128 * eb:128 * eb + 128])
                # mean over t via log2 tree of DVE adds (cols are t-major)
                for eb in range(4):
                    ms = SM.tile([128, 2048], bf, tag="ms")
                    nc.vector.tensor_tensor(ms[:], encTh[:, eb, 0:2048],
                                            encTh[:, eb, 2048:4096], op=AluOp.add)
                    for w in (1024, 512, 256, 128):
                        nc.vector.tensor_tensor(ms[:, 0:w], ms[:, 0:w],
                                                ms[:, w:2 * w], op=AluOp.add)
                    nc.vector.tensor_scalar(
                        meanT[:, eb, 128 * bth:128 * bth + 128], ms[:, 0:128],
                        1.0 / 32.0, None, op0=AluOp.mult)
                # enc_proj: ep[a, t, b] = W_enc^T @ encT
                for c in range(8):
                    for ab in range(2):
                        pp = PG.tile([128, 512], f32, tag="rz")
                        for eb in range(4):
                            nc.tensor.matmul(
                                pp[:],
                                w_enc[:, eb, 128 * ab:128 * ab + 128],
                                encTh[:, eb, 512 * c:512 * c + 512],
                                start=(eb == 0), stop=(eb == 3),
                            )
                        dst = ep[:, ab, 4 * c:4 * c + 4, 128 * bth:128 * bth + 128]
                        src2 = pp[:].rearrange("p (t b) -> p t b", b=128)
                        if ab == 0:
                            nc.vector.tensor_copy(dst, src2)
                        else:
                            nc.scalar.copy(dst, src2)

            # h0 = tanh(meanT @ ihw + ihb), feature-major
            for db in range(2):
                hp = PG.tile([128, BL], f32, tag="ng")
                for eb in range(4):
                    nc.tensor.matmul(hp[:], ihw[:, eb, 128 * db:128 * db + 128],
                                     meanT[:, eb, :], start=(eb == 0), stop=(eb == 3))
                nc.scalar.activation(hT[:, db, :], hp[:], ActF.Tanh,
                                     bias=ihb[:, db:db + 1])

            # ---------------- decode loop (2-half software pipeline) ----------------
            pending_oh = [None, None]  # ohB tiles awaiting transpose into onehotT
            for step in range(L):
                # dec_proj per half (fp8 w_dec x32)
                dps = []
                for half in range(2):
                    hs = 128 * half
                    dp = PG.tile([128, 2, 128], f32, tag="cp")
                    first = True
                    for db in range(2):
                        for ab in range(2):
                            nc.tensor.matmul(dp[:, ab, :],
                                             w_dec[:, db, 128 * ab:128 * ab + 128],
                                             hT[:, db, hs:hs + 128],
                                             start=first, stop=(db == 1 and ab == 1),
                                             skip_group_check=True)
                            first = False
                    dps.append(dp)

                # pending argmax transposes from previous step
                for half in range(2):
                    if pending_oh[half] is not None:
                        hs = 128 * half
                        tp2 = PB.tile([NCLS, 128], bf, tag="b")
                        nc.tensor.transpose(tp2[:], pending_oh[half][:], ident[:])
                        nc.vector.tensor_copy(onehotT[:, hs:hs + 128], tp2[:])
                        pending_oh[half] = None
                decTs = []
                for half in range(2):
                    decT = SM.tile([128, 2, 128], bf, tag=f"decT{half}")
                    nc.vector.tensor_scalar(decT[:], dps[half][:], INV, None,
                                            op0=AluOp.mult)
                    decTs.append(decT)

                # attention adds (DVE)
                stmps = []
                for half in range(2):
                    hs = 128 * half
                    stmp = SM.tile([128, 2, T, 128], bf, tag=f"stmp{half}")
                    for ab in range(2):
                        bcast = decTs[half][:, ab, :].rearrange(
                            "p (o b) -> p o b", o=1).broadcast_to([128, T, 128])
                        nc.vector.tensor_tensor(stmp[:, ab], ep[:, ab, :, hs:hs + 128],
                                                bcast, op=AluOp.add)
                    stmps.append(stmp)

                # GRU gate banks; early (emb + h) parts
                # rz bank cols: [r0, r1, z0, z1]; ng bank cols: [n0, n1, g0, g1]
                rzs, ngs = [None, None], [None, None]

                def gates_early(half):
                    hs = 128 * half
                    rz = PG.tile([128, 4, 128], f32, tag="rz")
                    ng = PG.tile([128, 4, 128], f32, tag="ng")
                    first = True
                    for ci in range(4):
                        fs = 128 * ci
                        nc.tensor.matmul(rz[:, ci, :], embW[:, fs:fs + 128],
                                         onehotT[:, hs:hs + 128],
                                         start=first, stop=False,
                                         skip_group_check=True)
                        first = False
                        for db in range(2):
                            nc.tensor.matmul(rz[:, ci, :], w_hhrz[:, db, fs:fs + 128],
                                             hT[:, db, hs:hs + 128],
                                             start=False, stop=False,
                                             skip_group_check=True)
                    first = True
                    for ci in range(2):
                        fs = 512 + 128 * ci
                        nc.tensor.matmul(ng[:, ci, :], embW[:, fs:fs + 128],
                                         onehotT[:, hs:hs + 128],
                                         start=first, stop=False,
                                         skip_group_check=True)
                        first = False
                    for ci in range(2):
                        nc.tensor.matmul(ng[:, 2 + ci, :], bhhn2[:, ci, :],
                                         onesN[:, 0:128], start=False, stop=False,
                                         skip_group_check=True)
                        for db in range(2):
                            nc.tensor.matmul(ng[:, 2 + ci, :],
                                             w_hhn2[:, db, 128 * ci:128 * ci + 128],
                                             hT[:, db, hs:hs + 128],
                                             start=False,
                                             stop=(ci == 1 and db == 1),
                                             skip_group_check=True)
                    rzs[half] = rz
                    ngs[half] = ng

                # attention phases, split for explicit ACT/PE queue ordering
                ebps, attnBs = {}, {}

                def a_tanh(half, ab):
                    hs = 128 * half
                    nc.scalar.activation(att[:, ab, :, hs:hs + 128],
                                         stmps[half][:, ab], ActF.Tanh)

                def a_energy(half):
                    hs = 128 * half
                    ebp = PB.tile([128, T], f32, tag="b")
                    for t in range(T):
                        for ab in range(2):
                            nc.tensor.matmul(ebp[:, t:t + 1],
                                             att[:, ab, t, hs:hs + 128],
                                             v_sb[:, ab, :],
                                             start=(t == 0 and ab == 0),
                                             stop=(t == T - 1 and ab == 1),
                                             skip_group_check=True)
                    ebps[half] = ebp

                def a_soft(half):
                    expB = SM.tile([128, T], bf, tag=f"expB{half}")
                    zc = SM.tile([128, 1], f32, tag=f"zc{half}")
                    nc.scalar.activation(expB[:], ebps[half][:], ActF.Exp,
                                         accum_out=zc[:])
                    rcp = SM.tile([128, 1], f32, tag=f"rcp{half}")
                    nc.vector.reciprocal(rcp[:], zc[:])
                    attnB = SM.tile([128, T], bf, tag=f"attnB{half}")
                    nc.vector.tensor_scalar(attnB[:], expB[:], rcp[:], -1.0 / T,
                                            op0=AluOp.mult, op1=AluOp.add)
                    tp = PB.tile([T, 128], bf, tag="b")
                    nc.tensor.transpose(tp[:], attnB[:], ident[:])
                    for bs in range(4):
                        nc.vector.tensor_copy(
                            attn_bd[32 * bs:32 * bs + 32,
                                    32 * half:32 * half + 32, bs],
                            tp[:, bs::4])

                def a_ctx(half):
                    hs = 128 * half
                    cp = PG.tile([128, 4, 128], f32, tag="cp")
                    first = True
                    for eb in range(4):
                        for gr in range(32):
                            g = 32 * half + gr
                            nc.tensor.matmul(cp[:, eb, 4 * gr:4 * gr + 4],
                                             enc_bd[:, g, 128 * eb:128 * eb + 128],
                                             attn_bd[:, g, :], start=first,
                                             stop=(eb == 3 and gr == 31),
                                             skip_group_check=True)
                            first = False
                        nc.vector.tensor_tensor(ctxT[:, eb, hs:hs + 128],
                                                cp[:, eb, :],
                                                meanT[:, eb, hs:hs + 128],
                                                op=AluOp.add)
                    for ci in range(4):
                        fs = 128 * ci
                        for eb in range(4):
                            nc.tensor.matmul(rzs[half][:, ci, :],
                                             w_ihc[:, eb, fs:fs + 128],
                                             ctxT[:, eb, hs:hs + 128],
                                             start=False,
                                             stop=(ci == 3 and eb == 3),
                                             skip_group_check=True)
                    for ci in range(2):
                        fs = 512 + 128 * ci
                        for eb in range(4):
                            nc.tensor.matmul(ngs[half][:, ci, :],
                                             w_ihc[:, eb, fs:fs + 128],
                                             ctxT[:, eb, hs:hs + 128],
                                             start=False,
                                             stop=(ci == 1 and eb == 3),
                                             skip_group_check=True)

                gates_early(0)
                a_tanh(0, 0)
                a_tanh(0, 1)
                a_energy(0)
                gates_early(1)
                a_tanh(1, 0)
                a_soft(0)
                a_tanh(1, 1)
                a_ctx(0)
                a_energy(1)
                a_soft(1)
                a_ctx(1)

                # fc1 ctx-part early: fills the GRU-tail PE window
                bankF = PG.tile([128, 2, BL], f32, tag="rz")
                first = True
                for dc in range(2):
                    ds = 128 * dc
                    for eb in range(4):
                        nc.tensor.matmul(bankF[:, dc, :], fc1c[:, eb, ds:ds + 128],
                                         ctxT[:, eb, :], start=first, stop=False,
                                         skip_group_check=True)
                        first = False

                # GRU elementwise tail: pass 1 (tr/tz + n-preact) both halves
                npres, tzs = [], []
                for half in range(2):
                    rz, ng = rzs[half], ngs[half]
                    tr_sb = SM.tile([128, 2, 128], bf, tag=f"tr{half}")
                    tz_sb = SM.tile([128, 2, 128], bf, tag=f"tz{half}")
                    for ci in range(2):
                        nc.scalar.activation(tr_sb[:, ci, :], rz[:, ci, :], ActF.Tanh,
                                             bias=bias_rz2[:, ci:ci + 1],
                                             scale=0.5 * INV)
                    for ci in range(2):
                        nc.scalar.activation(tz_sb[:, ci, :], rz[:, 2 + ci, :],
                                             ActF.Tanh,
                                             bias=bias_rz2[:, 2 + ci:3 + ci],
                                             scale=0.5 * INV)
                    u_sb = SM.tile([128, 2, 128], bf, tag=f"u{half}")
                    nc.vector.tensor_scalar(u_sb[:], ng[:, 2:4, :], INV, None,
                                            op0=AluOp.mult)
                    w1 = SM.tile([128, 2, 128], bf, tag=f"w1{half}")
                    nc.vector.scalar_tensor_tensor(w1[:], tr_sb[:], 1.0, u_sb[:],
                                                   op0=AluOp.add, op1=AluOp.mult)
                    npre = SM.tile([128, 2, 128], bf, tag=f"np{half}")
                    nc.vector.scalar_tensor_tensor(npre[:], ng[:, 0:2, :], INV, w1[:],
                                                   op0=AluOp.mult, op1=AluOp.add)
                    npres.append(npre)
                    tzs.append(tz_sb)
                # pass 2: n + h update both halves
                for half in range(2):
                    hs = 128 * half
                    n_sb = SM.tile([128, 2, 128], bf, tag=f"n{half}")
                    for ci in range(2):
                        nc.scalar.activation(n_sb[:, ci, :], npres[half][:, ci, :],
                                             ActF.Tanh, bias=bias_n[:, ci:ci + 1])
                    t1 = SM.tile([128, 2, 128], bf, tag=f"t1{half}")
                    nc.vector.tensor_tensor(t1[:], hT[:, :, hs:hs + 128], n_sb[:],
                                            op=AluOp.subtract)
                    t3 = SM.tile([128, 2, 128], bf, tag=f"t3{half}")
                    nc.vector.scalar_tensor_tensor(t3[:], tzs[half][:], 1.0, t1[:],
                                                   op0=AluOp.add, op1=AluOp.mult)
                    nc.vector.scalar_tensor_tensor(hT[:, :, hs:hs + 128], t3[:], 0.5,
                                                   n_sb[:],
                                                   op0=AluOp.mult, op1=AluOp.add)

                # fc1 h-part (closes the accumulation)
                for dc in range(2):
                    ds = 128 * dc
                    for db in range(2):
                        nc.tensor.matmul(bankF[:, dc, :], fc1h[:, db, ds:ds + 128],
                                         hT[:, db, :], start=False,
                                         stop=(dc == 1 and db == 1),
                                         skip_group_check=True)
                hidT = SM.tile([128, 2, BL], bf, tag="hidT")
                for dc in range(2):
                    nc.vector.tensor_scalar(hidT[:, dc, :], bankF[:, dc, :],
                                            fc1b[:, dc:dc + 1], 0.0,
                                            op0=AluOp.add, op1=AluOp.max)

                # fc2 + greedy argmax per half
                for half in range(2):
                    hs = 128 * half
                    lp = PB.tile([128, NCLS], f32, tag="b")
                    nc.tensor.matmul(lp[:], ones1[:], fc2b[:], start=True, stop=False,
                                     skip_group_check=True)
                    for db in range(2):
                        nc.tensor.matmul(lp[:], hidT[:, db, hs:hs + 128],
                                         fc2w[:, db, :],
                                         start=False, stop=(db == 1),
                                         skip_group_check=True)
                    nc.scalar.copy(out_sb[:, half, step, :], lp[:])
                    if step < L - 1:
                        sl = out_sb[:, half, step, :]
                        mx = SM.tile([128, 1], f32, tag=f"mx{half}")
                        nc.vector.tensor_reduce(mx[:], sl, axis=AxX, op=AluOp.max)
                        ohB = SM.tile([128, NCLS], bf, tag=f"oh{half}")
                        nc.vector.tensor_tensor(
                            ohB[:], sl, mx[:].broadcast_to([128, NCLS]),
                            op=AluOp.is_equal)
                        pending_oh[half] = ohB

            # ---------------- output DMA ----------------
            for half in range(2):
                nc.sync.dma_start(
                    d_out[128 * half:128 * half + 128],
                    out_sb[:, half, :, :],
                )

    nc.compile()
    return nc


def _get_nc():
    with _lock:
        if "nc" not in _cache:
            _cache["nc"] = _build()
        return _cache["nc"]


def kernel(**inputs):
    nc = _get_nc()
    from concourse.bass_utils import run_bass_kernel_spmd

    enc = np.ascontiguousarray(inputs["encoder_outputs"], dtype=np.float32)
    emb = inputs["emb"].astype(np.float32)
    W_enc = inputs["W_enc"].astype(np.float32)
    W_dec = inputs["W_dec"].astype(np.float32)
    v = inputs["v"].astype(np.float32)
    init_h_W = inputs["init_h_W"].astype(np.float32)
    init_h_b = inputs["init_h_b"].astype(np.float32)
    W_ih = inputs["W_ih"].astype(np.float32)
    b_ih = inputs["b_ih"].astype(np.float32)
    W_hh = inputs["W_hh"].astype(np.float32)
    b_hh = inputs["b_hh"].astype(np.float32)
    fc1_W = inputs["fc1_W"].astype(np.float32)
    fc1_b = inputs["fc1_b"].astype(np.float32)
    fc2_W = inputs["fc2_W"].astype(np.float32)
    fc2_b = inputs["fc2_b"].astype(np.float32)

    embW = emb @ W_ih[:EMB]  # [NCLS, 768]
    bias_rz = b_ih[:2 * DEC] + b_hh[:2 * DEC]

    bfc = lambda a: np.ascontiguousarray(a, dtype=_BF)
    f8c = lambda a: np.ascontiguousarray(np.asarray(a) * WS, dtype=_F8)
    f32c = lambda a: np.ascontiguousarray(a, dtype=np.float32)
    colview = lambda a, k: f32c(a.reshape(k, 128).T)
    shared = {
        "w_dec": f8c(W_dec),
        "w_enc": bfc(W_enc),
        "v": bfc(v.reshape(ATT, 1)),
        "embW": bfc(WS * embW),
        "w_ih_c": f8c(W_ih[EMB:]),
        "w_hh_rz": f8c(W_hh[:, :2 * DEC]),
        "w_hh_n2": f8c(0.5 * W_hh[:, 2 * DEC:]),
        "bias_rz2": colview(0.5 * bias_rz, 4),
        "bias_n": colview(b_ih[2 * DEC:], 2),
        "bhhn2": bfc((WS * 0.5 * b_hh[2 * DEC:]).reshape(1, 2, 128)),
        "fc1_w_h": bfc(fc1_W[:DEC]),
        "fc1_w_c": bfc(fc1_W[DEC:]),
        "fc1_b": colview(fc1_b, 2),
        "fc2_w": bfc(fc2_W),
        "fc2_b": bfc(fc2_b.reshape(1, NCLS)),
        "init_h_w": bfc(init_h_W),
        "init_h_b": colview(init_h_b, 2),
    }
    enc_bf = enc.astype(_BF)
    enc_f8 = enc.astype(_F8)
    in_maps = []
    for i in range(N_CORES):
        m = dict(shared)
        sh = enc_bf[i * BL:(i + 1) * BL]
        m["enc"] = np.ascontiguousarray(
            sh.reshape(2, 128, T, ENC).transpose(0, 2, 1, 3))
        # enc8 in bd layout: partition p = 32*(b%4) + t, free g = (bth, (b%128)//4)
        sh8 = enc_f8[i * BL:(i + 1) * BL]  # [256, 32, 512]
        m["enc8bd"] = np.ascontiguousarray(
            sh8.reshape(2, 32, 4, T, ENC).transpose(2, 3, 0, 1, 4).reshape(
                128, 64, ENC)).view(_BF)
        in_maps.append(m)

    res = run_bass_kernel_spmd(nc, in_maps, core_ids=list(range(N_CORES)),
                               trace=bool(int(os.environ.get("KTRACE", "0"))))
    out = np.concatenate([res.results[i]["out"] for i in range(N_CORES)], axis=0)
    kernel.last_results = res.results
    if bool(int(os.environ.get("KTRACE", "0"))):
        kernel.last_exec_time_ns = res.exec_time_ns
        kernel.last_profile = res.profile_json
    return out.astype(np.float32)


# revision 28
# speedup vs baseline: 1.0078x; 1.0063x over previous
"""Trainium2 Bass kernel for nn_AttentionDecoder (Bahdanau attention + GRU greedy decoder).

Sharding: pure data parallel, B=2048 split as 256 rows per core across 8 cores.

v4 design:
  - enc_bd / att / GRU+dec weights in fp8 E3M4 (weights pre-scaled x32 on host,
    un-scaled on PSUM evacuation): stationary LDWEIGHTS streams get fp8 FWL.
  - context mean-centered (attn - 1/32, meanT added on evacuation) so fp8 enc
    error applies only to the attention deviation.
  - the decode step is a 2-half software pipeline: attention, GRU gates,
    elementwise tail, fc2/argmax all emitted per 128-row half so each engine
    works one half while another engine works the other half.
  - PSUM banks host interleaved accumulation groups: only the first matmul of
    a bank per step uses start=True (clears the whole bank's has_written
    bits); later matmuls use start=False (first touch of a region overwrites,
    later touches accumulate).
  - biases ride ACT bias/scale operands (per-partition = per-feature);
    sigmoid via tanh(x/2); softmax without max-subtraction, Z via accum_out.
  - prologue: host supplies enc8 already in bd layout (one contiguous DMA);
    mean via a log2 tree of DVE adds; xbar transposes serial on the sync queue
    (concurrent transposes on both queues corrupt data).
"""

import os
import threading
import numpy as np
import ml_dtypes

N_CORES = 8
B, T, ENC = 2048, 32, 512
DEC, ATT, EMB, NCLS, L = 256, 256, 64, 37, 10
BL = B // N_CORES  # 256 per core
WS = 32.0  # fp8 weight pre-scale

_BF = ml_dtypes.bfloat16
_F8 = ml_dtypes.float8_e3m4

_lock = threading.Lock()
_cache = {}


def _build():
    import concourse.bass as bass
    import concourse.tile as tile
    from concourse import bacc, mybir

    bf = mybir.dt.bfloat16
    f8 = mybir.dt.float8e3
    f32 = mybir.dt.float32

    nc = bacc.Bacc("TRN2", target_bir_lowering=False, debug=False,
                   num_devices=N_CORES)

    # ---------------- DRAM inputs ----------------
    d_enc = nc.dram_tensor("enc", [2, T, 128, ENC], bf, kind="ExternalInput").ap()
    d_enc8 = nc.dram_tensor("enc8bd", [128, 64, ENC // 2], bf, kind="ExternalInput").ap()
    d_wdec = nc.dram_tensor("w_dec", [DEC, ATT], f8, kind="ExternalInput").ap()
    d_wenc = nc.dram_tensor("w_enc", [ENC, ATT], bf, kind="ExternalInput").ap()
    d_v = nc.dram_tensor("v", [ATT, 1], bf, kind="ExternalInput").ap()
    d_embW = nc.dram_tensor("embW", [NCLS, 3 * DEC], bf, kind="ExternalInput").ap()
    d_wihc = nc.dram_tensor("w_ih_c", [ENC, 3 * DEC], f8, kind="ExternalInput").ap()
    d_whhrz = nc.dram_tensor("w_hh_rz", [DEC, 2 * DEC], f8, kind="ExternalInput").ap()
    d_whhn2 = nc.dram_tensor("w_hh_n2", [DEC, DEC], f8, kind="ExternalInput").ap()
    d_bias_rz2 = nc.dram_tensor("bias_rz2", [128, 4], f32, kind="ExternalInput").ap()
    d_bias_n = nc.dram_tensor("bias_n", [128, 2], f32, kind="ExternalInput").ap()
    d_bhhn2 = nc.dram_tensor("bhhn2", [1, 2, 128], bf, kind="ExternalInput").ap()
    d_fc1h = nc.dram_tensor("fc1_w_h", [DEC, DEC], bf, kind="ExternalInput").ap()
    d_fc1c = nc.dram_tensor("fc1_w_c", [ENC, DEC], bf, kind="ExternalInput").ap()
    d_fc1b = nc.dram_tensor("fc1_b", [128, 2], f32, kind="ExternalInput").ap()
    d_fc2w = nc.dram_tensor("fc2_w", [DEC, NCLS], bf, kind="ExternalInput").ap()
    d_fc2b = nc.dram_tensor("fc2_b", [1, NCLS], bf, kind="ExternalInput").ap()
    d_ihw = nc.dram_tensor("init_h_w", [ENC, DEC], bf, kind="ExternalInput").ap()
    d_ihb = nc.dram_tensor("init_h_b", [128, 2], f32, kind="ExternalInput").ap()
    d_out = nc.dram_tensor("out", [BL, L, NCLS], f32, kind="ExternalOutput").ap()

    ident_np = np.eye(128, dtype=_BF)
    d_ident = nc.inline_tensor(ident_np, name="ident").ap()
    d_onesrow = nc.inline_tensor(np.ones((1, 256), dtype=_BF), name="onesrow").ap()

    AluOp = mybir.AluOpType
    ActF = mybir.ActivationFunctionType
    AxX = mybir.AxisListType.X
    INV = 1.0 / WS

    with tile.TileContext(nc) as tc:
        with (
            tc.tile_pool(name="persist", bufs=1) as P,
            tc.tile_pool(name="wpool", bufs=1) as W,
            tc.tile_pool(name="trans", bufs=1) as TR,
            tc.tile_pool(name="small", bufs=1) as SM,
            tc.tile_pool(name="psG", bufs=2, space="PSUM") as PG,
            tc.tile_pool(name="psB", bufs=2, space="PSUM") as PB,
        ):
            # ---------------- persistent SBUF ----------------
            enc_bd_raw = P.tile([128, 64, ENC // 2], bf, tag="enc_bd")  # fp8 bytes, 32KB/part
            ep = P.tile([128, 2, T, 256], bf, tag="ep")            # 32KB
            att = P.tile([128, 2, T, 256], f8, tag="att")          # 16KB
            hT = P.tile([128, 2, BL], bf, tag="hT")
            ctxT = P.tile([128, 4, BL], bf, tag="ctxT")
            meanT = P.tile([128, 4, BL], bf, tag="meanT")
            onehotT = P.tile([NCLS, BL], bf, tag="onehotT")
            attn_bd = P.tile([128, 64, 4], bf, tag="attn_bd")
            out_sb = P.tile([128, 2, L, NCLS], f32, tag="out_sb")

            # ---------------- weights ----------------
            def wload(tag, shape, src, rearr=None, q=nc.scalar, dt=bf):
                t = W.tile(shape, dt, tag=tag)
                q.dma_start(t[:], src if rearr is None else src.rearrange(rearr, p=128))
                return t

            w_dec = wload("w_dec", [128, 2, ATT], d_wdec, "(k p) n -> p k n", dt=f8)
            w_enc = wload("w_enc", [128, 4, ATT], d_wenc, "(k p) n -> p k n")
            v_sb = wload("v_sb", [128, 2, 1], d_v, "(k p) n -> p k n")
            embW = wload("embW", [NCLS, 3 * DEC], d_embW)
            w_ihc = wload("w_ihc", [128, 4, 3 * DEC], d_wihc, "(k p) n -> p k n",
                          dt=f8)
            w_hhrz = wload("w_hhrz", [128, 2, 2 * DEC], d_whhrz, "(k p) n -> p k n",
                           dt=f8)
            w_hhn2 = wload("w_hhn2", [128, 2, DEC], d_whhn2, "(k p) n -> p k n",
                           dt=f8)
            bhhn2 = wload("bhhn2", [1, 2, 128], d_bhhn2)
            fc1h = wload("fc1h", [128, 2, DEC], d_fc1h, "(k p) n -> p k n")
            fc1c = wload("fc1c", [128, 4, DEC], d_fc1c, "(k p) n -> p k n")
            fc2w = wload("fc2w", [128, 2, NCLS], d_fc2w, "(k p) n -> p k n")
            fc2b = wload("fc2b", [1, NCLS], d_fc2b)
            ihw = wload("ihw", [128, 4, DEC], d_ihw, "(k p) n -> p k n")
            ident = wload("ident", [128, 128], d_ident)
            ones1 = W.tile([1, 128], bf)
            nc.scalar.dma_start(ones1[:], d_onesrow[:, 0:128])
            onesN = W.tile([1, BL], bf)
            nc.scalar.dma_start(onesN[:], d_onesrow[:])

            def f32load(tag, shape, src, q=nc.scalar):
                t = W.tile(shape, f32, tag=tag)
                q.dma_start(t[:], src)
                return t

            bias_rz2 = f32load("bias_rz2", [128, 4], d_bias_rz2)
            bias_n = f32load("bias_n", [128, 2], d_bias_n)
            fc1b = f32load("fc1b", [128, 2], d_fc1b)
            ihb = f32load("ihb", [128, 2], d_ihb)

            # ---------------- prologue ----------------
            nc.vector.memset(onehotT[:], 0)
            nc.vector.memset(onehotT[0:1, :], 1.0)
            nc.vector.memset(attn_bd[:], 0)

            # enc_bd: host pre-laid-out, one contiguous DMA on the scalar queue
            for i in range(4):
                nc.scalar.dma_start(enc_bd_raw[:, 16 * i:16 * i + 16, :],
                                    d_enc8[:, 16 * i:16 * i + 16, :])
            enc_bd = enc_bd_raw[:].bitcast(f8)

            encTh = TR.tile([128, 4, 32 * 128], bf, tag="encTh", bufs=1)
            for bth in range(2):
                d_encb = d_enc[bth].rearrange("t b e -> (t b) e")
                for eb in range(4):
                    nc.sync.dma_start_transpose(encTh[:, eb, :],
                                                d_encb[:, 128 * eb:128 * eb + 128])
                # mean over t via log2 tree of DVE adds (cols are t-major)
                for eb in range(4):
                    ms = SM.tile([128, 2048], bf, tag="ms")
                    nc.vector.tensor_tensor(ms[:], encTh[:, eb, 0:2048],
                                            encTh[:, eb, 2048:4096], op=AluOp.add)
                    for w in (1024, 512, 256, 128):
                        nc.vector.tensor_tensor(ms[:, 0:w], ms[:, 0:w],
                                                ms[:, w:2 * w], op=AluOp.add)
                    nc.vector.tensor_scalar(
                        meanT[:, eb, 128 * bth:128 * bth + 128], ms[:, 0:128],
                        1.0 / 32.0, None, op0=AluOp.mult)
                # enc_proj: ep[a, t, b] = W_enc^T @ encT
                for c in range(8):
                    for ab in range(2):
                        pp = PG.tile([128, 512], f32, tag="rz")
                        for eb in range(4):
                            nc.tensor.matmul(
                                pp[:],
                                w_enc[:, eb, 128 * ab:128 * ab + 128],
                                encTh[:, eb, 512 * c:512 * c + 512],
                                start=(eb == 0), stop=(eb == 3),
                            )
                        dst = ep[:, ab, 4 * c:4 * c + 4, 128 * bth:128 * bth + 128]
                        src2 = pp[:].rearrange("p (t b) -> p t b", b=128)
                        if ab == 0:
                            nc.vector.tensor_copy(dst, src2)
                        else:
                            nc.scalar.copy(dst, src2)

            # h0 = tanh(meanT @ ihw + ihb), feature-major
            for db in range(2):
                hp = PG.tile([128, BL], f32, tag="ng")
                for eb in range(4):
                    nc.tensor.matmul(hp[:], ihw[:, eb, 128 * db:128 * db + 128],
                                     meanT[:, eb, :], start=(eb == 0), stop=(eb == 3))
                nc.scalar.activation(hT[:, db, :], hp[:], ActF.Tanh,
                                     bias=ihb[:, db:db + 1])

            # ---------------- decode loop (2-half software pipeline) ----------------
            pending_oh = [None, None]  # ohB tiles awaiting transpose into onehotT
            for step in range(L):
                # dec_proj per half (fp8 w_dec x32)
                dps = []
                for half in range(2):
                    hs = 128 * half
                    dp = PG.tile([128, 2, 128], f32, tag="cp")
                    first = True
                    for db in range(2):
                        for ab in range(2):
                            nc.tensor.matmul(dp[:, ab, :],
                                             w_dec[:, db, 128 * ab:128 * ab + 128],
                                             hT[:, db, hs:hs + 128],
                                             start=first, stop=(db == 1 and ab == 1),
                                             skip_group_check=True)
                            first = False
                    dps.append(dp)

                # pending argmax transposes from previous step
                for half in range(2):
                    if pending_oh[half] is not None:
                        hs = 128 * half
                        tp2 = PB.tile([NCLS, 128], bf, tag="b")
                        nc.tensor.transpose(tp2[:], pending_oh[half][:], ident[:])
                        nc.vector.tensor_copy(onehotT[:, hs:hs + 128], tp2[:])
                        pending_oh[half] = None
                decTs = []
                for half in range(2):
                    decT = SM.tile([128, 2, 128], bf, tag=f"decT{half}")
                    nc.vector.tensor_scalar(decT[:], dps[half][:], INV, None,
                                            op0=AluOp.mult)
                    decTs.append(decT)

                # attention adds (DVE)
                stmps = []
                for half in range(2):
                    hs = 128 * half
                    stmp = SM.tile([128, 2, T, 128], bf, tag=f"stmp{half}")
                    for ab in range(2):
                        bcast = decTs[half][:, ab, :].rearrange(
                            "p (o b) -> p o b", o=1).broadcast_to([128, T, 128])
                        nc.vector.tensor_tensor(stmp[:, ab], ep[:, ab, :, hs:hs + 128],
                                                bcast, op=AluOp.add)
                    stmps.append(stmp)

                # GRU gate banks; early (emb + h) parts
                # rz bank cols: [r0, r1, z0, z1]; ng bank cols: [n0, n1, g0, g1]
                rzs, ngs = [None, None], [None, None]

                def gates_early(half):
                    hs = 128 * half
                    rz = PG.tile([128, 4, 128], f32, tag="rz")
                    ng = PG.tile([128, 4, 128], f32, tag="ng")
                    first = True
                    for ci in range(4):
                        fs = 128 * ci
                        nc.tensor.matmul(rz[:, ci, :], embW[:, fs:fs + 128],
                                         onehotT[:, hs:hs + 128],
                                         start=first, stop=False,
                                         skip_group_check=True)
                        first = False
                        for db in range(2):
                            nc.tensor.matmul(rz[:, ci, :], w_hhrz[:, db, fs:fs + 128],
                                             hT[:, db, hs:hs + 128],
                                             start=False, stop=False,
                                             skip_group_check=True)
                    first = True
                    for ci in range(2):
                        fs = 512 + 128 * ci
                        nc.tensor.matmul(ng[:, ci, :], embW[:, fs:fs + 128],
                                         onehotT[:, hs:hs + 128],
                                         start=first, stop=False,
                                         skip_group_check=True)
                        first = False
                    for ci in range(2):
                        nc.tensor.matmul(ng[:, 2 + ci, :], bhhn2[:, ci, :],
                                         onesN[:, 0:128], start=False, stop=False,
                                         skip_group_check=True)
                        for db in range(2):
                            nc.tensor.matmul(ng[:, 2 + ci, :],
                                             w_hhn2[:, db, 128 * ci:128 * ci + 128],
                                             hT[:, db, hs:hs + 128],
                                             start=False,
                                             stop=(ci == 1 and db == 1),
                                             skip_group_check=True)
                    rzs[half] = rz
                    ngs[half] = ng

                # attention phases, split for explicit ACT/PE queue ordering
                ebps, attnBs = {}, {}

                def a_tanh(half, ab):
                    hs = 128 * half
                    nc.scalar.activation(att[:, ab, :, hs:hs + 128],
                                         stmps[half][:, ab], ActF.Tanh)

                def a_energy(half):
                    hs = 128 * half
                    ebp = PB.tile([128, T], f32, tag="b")
                    for t in range(T):
                        for ab in range(2):
                            nc.tensor.matmul(ebp[:, t:t + 1],
                                             att[:, ab, t, hs:hs + 128],
                                             v_sb[:, ab, :],
                                             start=(t == 0 and ab == 0),
                                             stop=(t == T - 1 and ab == 1),
                                             skip_group_check=True)
                    ebps[half] = ebp

                def a_soft(half):
                    expB = SM.tile([128, T], bf, tag=f"expB{half}")
                    zc = SM.tile([128, 1], f32, tag=f"zc{half}")
                    nc.scalar.activation(expB[:], ebps[half][:], ActF.Exp,
                                         accum_out=zc[:])
                    rcp = SM.tile([128, 1], f32, tag=f"rcp{half}")
                    nc.vector.reciprocal(rcp[:], zc[:])
                    attnB = SM.tile([128, T], bf, tag=f"attnB{half}")
                    nc.vector.tensor_scalar(attnB[:], expB[:], rcp[:], -1.0 / T,
                                            op0=AluOp.mult, op1=AluOp.add)
                    tp = PB.tile([T, 128], bf, tag="b")
                    nc.tensor.transpose(tp[:], attnB[:], ident[:])
                    for bs in range(4):
                        nc.vector.tensor_copy(
                            attn_bd[32 * bs:32 * bs + 32,
                                    32 * half:32 * half + 32, bs],
                            tp[:, bs::4])

                def a_ctx(half):
                    hs = 128 * half
                    cp = PG.tile([128, 4, 128], f32, tag="cp")
                    first = True
                    for eb in range(4):
                        for gr in range(32):
                            g = 32 * half + gr
                            nc.tensor.matmul(cp[:, eb, 4 * gr:4 * gr + 4],
                                             enc_bd[:, g, 128 * eb:128 * eb + 128],
                                             attn_bd[:, g, :], start=first,
                                             stop=(eb == 3 and gr == 31),
                                             skip_group_check=True)
                            first = False
                        nc.vector.tensor_tensor(ctxT[:, eb, hs:hs + 128],
                                                cp[:, eb, :],
                                                meanT[:, eb, hs:hs + 128],
                                                op=AluOp.add)
                    for ci in range(4):
                        fs = 128 * ci
                        for eb in range(4):
                            nc.tensor.matmul(rzs[half][:, ci, :],
                                             w_ihc[:, eb, fs:fs + 128],
                                             ctxT[:, eb, hs:hs + 128],
                                             start=False,
                                             stop=(ci == 3 and eb == 3),
                                             skip_group_check=True)
                    for ci in range(2):
                        fs = 512 + 128 * ci
                        for eb in range(4):
                            nc.tensor.matmul(ngs[half][:, ci, :],
                                             w_ihc[:, eb, fs:fs + 128],
                                             ctxT[:, eb, hs:hs + 128],
                                             start=False,
                                             stop=(ci == 1 and eb == 3),
                                             skip_group_check=True)

                gates_early(0)
                a_tanh(0, 0)
                a_tanh(0, 1)
                a_energy(0)
                gates_early(1)
                a_tanh(1, 0)
                a_soft(0)
                a_tanh(1, 1)
                a_ctx(0)

                # GRU elementwise tail, per-half pieces
                npres, tzs = [None, None], [None, None]

                def pass1(half):
                    rz, ng = rzs[half], ngs[half]
                    tr_sb = SM.tile([128, 2, 128], bf, tag=f"tr{half}")
                    tz_sb = SM.tile([128, 2, 128], bf, tag=f"tz{half}")
                    for ci in range(2):
                        nc.scalar.activation(tr_sb[:, ci, :], rz[:, ci, :], ActF.Tanh,
                                             bias=bias_rz2[:, ci:ci + 1],
                                             scale=0.5 * INV)
                    for ci in range(2):
                        nc.scalar.activation(tz_sb[:, ci, :], rz[:, 2 + ci, :],
                                             ActF.Tanh,
                                             bias=bias_rz2[:, 2 + ci:3 + ci],
                                             scale=0.5 * INV)
                    u_sb = SM.tile([128, 2, 128], bf, tag=f"u{half}")
                    nc.vector.tensor_scalar(u_sb[:], ng[:, 2:4, :], INV, None,
                                            op0=AluOp.mult)
                    w1 = SM.tile([128, 2, 128], bf, tag=f"w1{half}")
                    nc.vector.scalar_tensor_tensor(w1[:], tr_sb[:], 1.0, u_sb[:],
                                                   op0=AluOp.add, op1=AluOp.mult)
                    npre = SM.tile([128, 2, 128], bf, tag=f"np{half}")
                    nc.vector.scalar_tensor_tensor(npre[:], ng[:, 0:2, :], INV, w1[:],
                                                   op0=AluOp.mult, op1=AluOp.add)
                    npres[half] = npre
                    tzs[half] = tz_sb

                def pass2(half):
                    hs = 128 * half
                    n_sb = SM.tile([128, 2, 128], bf, tag=f"n{half}")
                    for ci in range(2):
                        nc.scalar.activation(n_sb[:, ci, :], npres[half][:, ci, :],
                                             ActF.Tanh, bias=bias_n[:, ci:ci + 1])
                    t1 = SM.tile([128, 2, 128], bf, tag=f"t1{half}")
                    nc.vector.tensor_tensor(t1[:], hT[:, :, hs:hs + 128], n_sb[:],
                                            op=AluOp.subtract)
                    t3 = SM.tile([128, 2, 128], bf, tag=f"t3{half}")
                    nc.vector.scalar_tensor_tensor(t3[:], tzs[half][:], 1.0, t1[:],
                                                   op0=AluOp.add, op1=AluOp.mult)
                    nc.vector.scalar_tensor_tensor(hT[:, :, hs:hs + 128], t3[:], 0.5,
                                                   n_sb[:],
                                                   op0=AluOp.mult, op1=AluOp.add)

                pass1(0)
                a_energy(1)
                a_soft(1)
                pass2(0)
                a_ctx(1)
                pass1(1)

                # fc1 ctx-part early: fills the GRU-tail PE window
                bankF = PG.tile([128, 2, BL], f32, tag="rz")
                first = True
                for dc in range(2):
                    ds = 128 * dc
                    for eb in range(4):
                        nc.tensor.matmul(bankF[:, dc, :], fc1c[:, eb, ds:ds + 128],
                                         ctxT[:, eb, :], start=first, stop=False,
                                         skip_group_check=True)
                        first = False


                pass2(1)

                # fc1 h-part (closes the accumulation)
                for dc in range(2):
                    ds = 128 * dc
                    for db in range(2):
                        nc.tensor.matmul(bankF[:, dc, :], fc1h[:, db, ds:ds + 128],
                                         hT[:, db, :], start=False,
                                         stop=(dc == 1 and db == 1),
                                         skip_group_check=True)
                hidT = SM.tile([128, 2, BL], bf, tag="hidT")
                for dc in range(2):
                    nc.vector.tensor_scalar(hidT[:, dc, :], bankF[:, dc, :],
                                            fc1b[:, dc:dc + 1], 0.0,
                                            op0=AluOp.add, op1=AluOp.max)

                # fc2 + greedy argmax per half
                for half in range(2):
                    hs = 128 * half
                    lp = PB.tile([128, NCLS], f32, tag="b")
                    nc.tensor.matmul(lp[:], ones1[:], fc2b[:], start=True, stop=False,
                                     skip_group_check=True)
                    for db in range(2):
                        nc.tensor.matmul(lp[:], hidT[:, db, hs:hs + 128],
                                         fc2w[:, db, :],
                                         start=False, stop=(db == 1),
                                         skip_group_check=True)
                    nc.scalar.copy(out_sb[:, half, step, :], lp[:])
                    if step < L - 1:
                        sl = out_sb[:, half, step, :]
                        mx = SM.tile([128, 1], f32, tag=f"mx{half}")
                        nc.vector.tensor_reduce(mx[:], sl, axis=AxX, op=AluOp.max)
                        ohB = SM.tile([128, NCLS], bf, tag=f"oh{half}")
                        nc.vector.tensor_tensor(
                            ohB[:], sl, mx[:].broadcast_to([128, NCLS]),
                            op=AluOp.is_equal)
                        pending_oh[half] = ohB

            # ---------------- output DMA ----------------
            for half in range(2):
                nc.sync.dma_start(
                    d_out[128 * half:128 * half + 128],
                    out_sb[:, half, :, :],
                )

    nc.compile()
    return nc


def _get_nc():
    with _lock:
        if "nc" not in _cache:
            _cache["nc"] = _build()
        return _cache["nc"]


def kernel(**inputs):
    nc = _get_nc()
    from concourse.bass_utils import run_bass_kernel_spmd

    enc = np.ascontiguousarray(inputs["encoder_outputs"], dtype=np.float32)
    emb = inputs["emb"].astype(np.float32)
    W_enc = inputs["W_enc"].astype(np.float32)
    W_dec = inputs["W_dec"].astype(np.float32)
    v = inputs["v"].astype(np.float32)
    init_h_W = inputs["init_h_W"].astype(np.float32)
    init_h_b = inputs["init_h_b"].astype(np.float32)
    W_ih = inputs["W_ih"].astype(np.float32)
    b_ih = inputs["b_ih"].astype(np.float32)
    W_hh = inputs["W_hh"].astype(np.float32)
    b_hh = inputs["b_hh"].astype(np.float32)
    fc1_W = inputs["fc1_W"].astype(np.float32)
    fc1_b = inputs["fc1_b"].astype(np.float32)
    fc2_W = inputs["fc2_W"].astype(np.float32)
    fc2_b = inputs["fc2_b"].astype(np.float32)

    embW = emb @ W_ih[:EMB]  # [NCLS, 768]
    bias_rz = b_ih[:2 * DEC] + b_hh[:2 * DEC]

    bfc = lambda a: np.ascontiguousarray(a, dtype=_BF)
    f8c = lambda a: np.ascontiguousarray(np.asarray(a) * WS, dtype=_F8)
    f32c = lambda a: np.ascontiguousarray(a, dtype=np.float32)
    colview = lambda a, k: f32c(a.reshape(k, 128).T)
    shared = {
        "w_dec": f8c(W_dec),
        "w_enc": bfc(W_enc),
        "v": bfc(v.reshape(ATT, 1)),
        "embW": bfc(WS * embW),
        "w_ih_c": f8c(W_ih[EMB:]),
        "w_hh_rz": f8c(W_hh[:, :2 * DEC]),
        "w_hh_n2": f8c(0.5 * W_hh[:, 2 * DEC:]),
        "bias_rz2": colview(0.5 * bias_rz, 4),
        "bias_n": colview(b_ih[2 * DEC:], 2),
        "bhhn2": bfc((WS * 0.5 * b_hh[2 * DEC:]).reshape(1, 2, 128)),
        "fc1_w_h": bfc(fc1_W[:DEC]),
        "fc1_w_c": bfc(fc1_W[DEC:]),
        "fc1_b": colview(fc1_b, 2),
        "fc2_w": bfc(fc2_W),
        "fc2_b": bfc(fc2_b.reshape(1, NCLS)),
        "init_h_w": bfc(init_h_W),
        "init_h_b": colview(init_h_b, 2),
    }
    enc_bf = enc.astype(_BF)
    enc_f8 = enc.astype(_F8)
    in_maps = []
    for i in range(N_CORES):
        m = dict(shared)
        sh = enc_bf[i * BL:(i + 1) * BL]
        m["enc"] = np.ascontiguousarray(
            sh.reshape(2, 128, T, ENC).transpose(0, 2, 1, 3))
        # enc8 in bd layout: partition p = 32*(b%4) + t, free g = (bth, (b%128)//4)
        sh8 = enc_f8[i * BL:(i + 1) * BL]  # [256, 32, 512]
        m["enc8bd"] = np.ascontiguousarray(
            sh8.reshape(2, 32, 4, T, ENC).transpose(2, 3, 0, 1, 4).reshape(
                128, 64, ENC)).view(_BF)
        in_maps.append(m)

    res = run_bass_kernel_spmd(nc, in_maps, core_ids=list(range(N_CORES)),
                               trace=bool(int(os.environ.get("KTRACE", "0"))))
    out = np.concatenate([res.results[i]["out"] for i in range(N_CORES)], axis=0)
    kernel.last_results = res.results
    if bool(int(os.environ.get("KTRACE", "0"))):
        kernel.last_exec_time_ns = res.exec_time_ns
        kernel.last_profile = res.profile_json
    return out.astype(np.float32)
